# revision 56
# baseline (speedup 1.0000x reference)
"""Causal self-attention (single head, S=4096, D=1024) on 8 TRN2 NeuronCores.

Strategy (striped sequence-parallel + split-fp8 DoubleRow matmuls):
  - Core c owns the strided query rows {i : i mod 8 == c} (local index
    l = 0..511, global i = 8l + c) and computes K/V projections for the
    contiguous rows [512c, 512(c+1)); K^T/V are AllGathered.
  - Every matmul runs in fp8e4 (e4m3) with DoubleRow perf mode, which packs
    TWO 128-deep contraction slices per instruction at 0.5 PE cycles/row.
    Each operand is carried as an (hi, lo) e4m3 pair (lo = exact residual of
    the hi quantization); a product (A_hi+A_lo)(B_hi+B_lo) is evaluated as
    hi*hi + hi*lo + lo*hi (the lo*lo term is ~1e-3 relative and dropped).
    That is 3 slice-products per pair = 1.5 DoubleRow instructions per
    128-slice, i.e. 0.75x the PE cycles of bf16 at ~bf16 accuracy.
  - W is pre-scaled by 32 on the host (W' ~ N(0,1)) so its fp8 residual
    stays in e4m3's normal range; 1/1024 folds into the softmax scale and
    the extra 32x on V divides out on the host.
  - exp() runs with a -2.0 bias so unnormalized scores stay below e4m3's
    448 max (bias cancels in the softmax normalization).
  - Scores are computed TRANSPOSED (S^T[j, l], keys on partitions) with a
    per-js causal trim: key block js of window W only attends local queries
    l >= 64W + 16js, which makes the mask band a single [128 x 16] tile
    shared by every (W, js).
  - kT projection consumes its operands ko-pair-major so the PE starts as
    soon as the first weight pair lands (the head is DMA-paced); row sums
    accumulate in a dedicated PSUM bank as one long-lived group; quantize
    work is spread across Act/DVE/GpSimd; the PE stream is software-
    pipelined (scores of window W+1 before PV of window W).
"""

import numpy as np
import ml_dtypes

S = 4096
D = 1024
N_CORES = 8
P = 128
L = 512               # local query rows per core (striped)
N_WIN = 8
KT_ELEMS = P * 8 * 2 * L      # per-rank kT block: [dp 128][ko 8][role 2][j 512]
V_ELEMS = P * 4 * 2 * D       # per-rank v block: [jp 128][js 4][role 2][d 1024]
SCALE2 = 1.0 / (32.0 * 1024.0)   # 1/sqrt(D) / (32*32) from W pre-scaling
EXP_BIAS = -2.0
BIG_NEG = -1e30

_CACHE = {}


def _build(parts=frozenset({'sc', 'pv', 'act', 'msk', 'rs', 'dma', 'acc', 'ag'})):
    import concourse.bass as bass
    import concourse.mybir as mybir
    import concourse.tile as tile
    from concourse import bacc

    bf16 = mybir.dt.bfloat16
    f32 = mybir.dt.float32
    f8 = mybir.dt.float8e4
    DR = mybir.MatmulPerfMode.DoubleRow

    nc = bacc.Bacc("TRN2", target_bir_lowering=False, debug=False,
                   num_devices=N_CORES)

    # ---- per-core I/O (all fp8 operands are (hi, lo) e4m3 pairs) ----
    wq = nc.dram_tensor("wq", [P, 8, 2, D], f8, kind="ExternalInput")
    wk = nc.dram_tensor("wk", [P, 8, 2, D], f8, kind="ExternalInput")
    wv = nc.dram_tensor("wv", [P, 8, 2, D], f8, kind="ExternalInput")
    xkv = nc.dram_tensor("xkv", [P, 8, 2, L], f8, kind="ExternalInput")
    xq = nc.dram_tensor("xq", [P, 8, 2, L], f8, kind="ExternalInput")
    maskd = nc.dram_tensor("mask", [P, 16], f32, kind="ExternalInput")
    identd = nc.dram_tensor("ident", [P, P], mybir.dt.float32r,
                            kind="ExternalInput")
    outd = nc.dram_tensor("out", [L, D], bf16, kind="ExternalOutput")

    agin_k = nc.dram_tensor("agin_k", [1, KT_ELEMS], f8)
    agout_k = nc.dram_tensor("agout_k", [1, N_CORES * KT_ELEMS], f8,
                             addr_space="Shared")
    agin_v = nc.dram_tensor("agin_v", [1, V_ELEMS], f8)
    agout_v = nc.dram_tensor("agout_v", [1, N_CORES * V_ELEMS], f8,
                             addr_space="Shared")

    PRODUCTS = ((0, 0), (0, 1), (1, 0))   # (hi,hi), (hi,lo), (lo,hi)

    def ag(agin, agout):
        if 'ag' in parts:
            nc.gpsimd.collective_compute(
                "AllGather", mybir.AluOpType.bypass,
                replica_groups=[list(range(N_CORES))],
                ins=[agin.ap().opt()],
                outs=[agout.ap().opt()],
            )
        else:
            # Local stand-in with the same per-core traffic shape.
            n = agin.shape[1]
            for sp in range(2):
                off = sp * (n // 2)
                nc.sync.dma_start(
                    bass.AP(agout, off, [[1, 1], [1, n // 2]]),
                    bass.AP(agin, off, [[1, 1], [1, n // 2]]))

    with tile.TileContext(nc) as tc:
        with tc.tile_pool(name="wpool", bufs=12) as wpool, \
             tc.tile_pool(name="xpool", bufs=8) as xpool, \
             tc.tile_pool(name="qt", bufs=1) as qtpool, \
             tc.tile_pool(name="stage", bufs=3) as stage, \
             tc.tile_pool(name="consts", bufs=1) as consts, \
             tc.tile_pool(name="accs", bufs=1) as accs:

            # ---------------- Phase 1: projections ----------------
            # per ko-pair tiles so the PE can start on pair 0 immediately
            xkvp = [xpool.tile([P, 2, 2, L], f8, name=f"xkv{i}", tag="x")
                    for i in range(4)]
            xqp = [xpool.tile([P, 2, 2, L], f8, name=f"xq{i}", tag="x")
                   for i in range(4)]
            wkp = [wpool.tile([P, 2, 2, D], f8, name=f"wk{i}", tag="w")
                   for i in range(4)]
            wvp = [wpool.tile([P, 2, 2, D], f8, name=f"wv{i}", tag="w")
                   for i in range(4)]
            wqp = [wpool.tile([P, 2, 2, D], f8, name=f"wq{i}", tag="w")
                   for i in range(4)]
            bias_sb = consts.tile([P, 1], f32, name="bias_sb")
            id_sb = consts.tile([P, P], mybir.dt.float32r, name="id_sb")
            mask_sb = consts.tile([P, 16], f32, name="mask_sb")
            ones_t = consts.tile([P, 4, 1], f8, name="ones_t")
            nc.vector.memset(bias_sb[:], EXP_BIAS)
            nc.vector.memset(ones_t[:], 1.0)
            # DMA order follows consumption: kT path (hi roles of the first
            # pair land first so the PE starts sooner), then V, then Q
            for i in range(4):
                nc.sync.dma_start(xkvp[i][:], xkv[:, 2 * i:2 * i + 2])
                nc.sync.dma_start(wkp[i][:], wk[:, 2 * i:2 * i + 2])
            for i in range(4):
                nc.sync.dma_start(wvp[i][:], wv[:, 2 * i:2 * i + 2])
                nc.sync.dma_start(xqp[i][:], xq[:, 2 * i:2 * i + 2])
            for i in range(4):
                nc.sync.dma_start(wqp[i][:], wq[:, 2 * i:2 * i + 2])
            nc.sync.dma_start(mask_sb[:], maskd[:])
            nc.sync.dma_start(id_sb[:], identd[:])

            def quant(hi_out, lo_out, src, sub_eng=None):
                # src (f32) -> hi e4m3 + exact-residual lo e4m3
                nc.scalar.activation(hi_out, src,
                                     mybir.ActivationFunctionType.Copy)
                (sub_eng or nc.vector).tensor_sub(lo_out, src, hi_out)

            halves2 = [(0, 512), (512, 1024)]

            # ---- kT projection (dt2-major; quants drain incrementally) ----
            with tc.tile_pool(name="pps", bufs=2, space="PSUM") as pps, \
                 tc.tile_pool(name="ppsv", bufs=4, space="PSUM") as ppsv:
                for dt2 in range(4):
                    ps = pps.tile([P, 2, L], f32, name=f"kt_ps{dt2}",
                                  tag="ktps")
                    for h in range(2):
                        d0 = (dt2 * 2 + h) * P
                        first = True
                        for kp in range(4):
                            for pi, (ra, rb) in enumerate(PRODUCTS):
                                nc.tensor.matmul(
                                    ps[:, h, :],
                                    wkp[kp][:, :, ra, d0:d0 + P],
                                    xkvp[kp][:, :, rb, :],
                                    start=first,
                                    stop=(kp == 3 and pi == 2),
                                    perf_mode=DR)
                                first = False
                    st = stage.tile([P, 2, 2, L], f8, name=f"kt_st{dt2}",
                                    tag="ktst")
                    for h in range(2):
                        quant(st[:, h, 0, :], st[:, h, 1, :], ps[:, h, :])
                    # agin_k layout: [dp][ko][role][j]
                    dst = bass.AP(agin_k, dt2 * 2 * 2 * L,
                                  [[8 * 2 * L, P], [2 * L, 2], [L, 2], [1, L]])
                    nc.sync.dma_start(dst, st[:])

                # ---- AllGather K (early, overlaps V/Q projections) ----
                ag(agin_k, agout_k)

                # ---- V projection (st_i-major; wv arrives during kT) ----
                for st_i in range(4):
                    st = stage.tile([P, 2, D], f8, name=f"v_st{st_i}",
                                    tag="vst")
                    for a, b in halves2:
                        # one single-bank tile per 512-col half: slots free
                        # as soon as that half's quant drains, so the next
                        # group's matmuls never wait on a whole-tile quant
                        ps = ppsv.tile([P, 512], f32,
                                       name=f"v_ps{st_i}_{a}", tag="vps")
                        for kp in range(4):
                            for pi, (ra, rb) in enumerate(PRODUCTS):
                                nc.tensor.matmul(
                                    ps[:],
                                    xkvp[kp][:, :, ra,
                                             st_i * P:(st_i + 1) * P],
                                    wvp[kp][:, :, rb, a:b],
                                    start=(kp == 0 and pi == 0),
                                    stop=(kp == 3 and pi == 2),
                                    perf_mode=DR)
                        quant(st[:, 0, a:b], st[:, 1, a:b], ps[:])
                    # agin_v layout: [jp][js][role][d]
                    dst = bass.AP(agin_v, st_i * 2 * D,
                                  [[4 * 2 * D, P], [D, 2], [1, D]])
                    nc.sync.dma_start(dst, st[:])

                ag(agin_v, agout_v)

                # ---- Q projection (dt2-major) -> qT stays in SBUF ----
                qt8 = qtpool.tile([P, 8, 2, L], f8, name="qt8")
                for dt2 in range(4):
                    for h in range(2):
                        ps = pps.tile([P, L], f32, name=f"q_ps{dt2}_{h}",
                                      tag="ktps")
                        d0 = (dt2 * 2 + h) * P
                        first = True
                        for kp in range(4):
                            for pi, (ra, rb) in enumerate(PRODUCTS):
                                nc.tensor.matmul(
                                    ps[:],
                                    wqp[kp][:, :, ra, d0:d0 + P],
                                    xqp[kp][:, :, rb, :],
                                    start=first,
                                    stop=(kp == 3 and pi == 2),
                                    perf_mode=DR)
                                first = False
                        quant(qt8[:, 2 * dt2 + h, 0, :],
                              qt8[:, 2 * dt2 + h, 1, :], ps[:])

            # ---------------- Phase 2: attention ----------------
            # f32r so the PE identity-fold can consume it directly
            acc = accs.tile([P, 4, D], mybir.dt.float32r, name="acc")
            recip_sb = consts.tile([P, 4], f32, name="recip_sb")

            with tc.tile_pool(name="ktw", bufs=3) as ktw, \
                 tc.tile_pool(name="vw", bufs=4) as vw, \
                 tc.tile_pool(name="pt", bufs=2) as ptp, \
                 tc.tile_pool(name="et", bufs=4) as etp, \
                 tc.tile_pool(name="outp", bufs=2) as outp, \
                 tc.tile_pool(name="pvps", bufs=4, space="PSUM") as pvps, \
                 tc.tile_pool(name="rsps", bufs=1, space="PSUM") as rsps, \
                 tc.tile_pool(name="stps", bufs=3, space="PSUM") as stps:

                # every window's row-sum matmuls accumulate here as ONE
                # long-lived group in a dedicated PSUM bank
                rs_ps = rsps.tile([P, 4], f32, name="rs_ps")

                kt_tiles = {}
                v_tiles = {}
                pt_tiles = {}
                pv_pair = {}
                o_tiles = {}

                def dma_win_kt(W):
                    kt_w = ktw.tile([P, 8, 2, L], f8, name=f"kt_w{W}", tag="ktw")
                    for sp in range(2):
                        nc.sync.dma_start(
                            kt_w[:, sp * 4:(sp + 1) * 4],
                            bass.AP(agout_k,
                                    W * KT_ELEMS + sp * 4 * 2 * L,
                                    [[8 * 2 * L, P], [2 * L, 4], [L, 2],
                                     [1, L]]))
                    kt_tiles[W] = kt_w

                def dma_win_v(W):
                    v_w = vw.tile([P, 4, 2, D], f8, name=f"v_w{W}", tag="vw")
                    for sp in range(2):
                        nc.sync.dma_start(
                            v_w[:, sp * 2:(sp + 1) * 2],
                            bass.AP(agout_v,
                                    W * V_ELEMS + sp * 2 * 2 * D,
                                    [[4 * 2 * D, P], [2 * D, 2], [D, 2],
                                     [1, D]]))
                    v_tiles[W] = v_w

                def scores(W):
                    l0 = 64 * W
                    lc0 = 128 * (W // 2)
                    pt = ptp.tile([P, 4, 2, L], f8, name=f"pt{W}", tag="pt")
                    kt_w = kt_tiles[W]
                    for js in range(4):
                        ljs = l0 + 16 * js
                        if ljs != lc0:
                            # zero the strip PV touches but scores skip
                            nc.vector.memset(pt[:, js, :, lc0:ljs], 0.0)
                        lcnt = L - ljs
                        st_ps = stps.tile([P, 512], f32, name=f"st{W}_{js}",
                                          tag="st")
                        if 'sc' in parts:
                            first = True
                            for kp in range(4):
                                for pi, (ra, rb) in enumerate(PRODUCTS):
                                    nc.tensor.matmul(
                                        st_ps[:, 0:lcnt],
                                        kt_w[:, 2 * kp:2 * kp + 2, ra,
                                             js * P:(js + 1) * P],
                                        qt8[:, 2 * kp:2 * kp + 2, rb,
                                            ljs:L],
                                        start=first,
                                        stop=(kp == 3 and pi == 2),
                                        perf_mode=DR)
                                    first = False
                        else:
                            nc.vector.memset(st_ps[:, 0:lcnt], 0.1)
                        if 'msk' in parts:
                            nc.vector.tensor_add(st_ps[:, 0:16],
                                                 st_ps[:, 0:16], mask_sb[:])
                        et = etp.tile([P, 512], f32, name=f"et{W}_{js}",
                                      tag="et")
                        if 'act' in parts:
                            nc.scalar.activation(
                                et[:, 0:lcnt], st_ps[:, 0:lcnt],
                                mybir.ActivationFunctionType.Exp,
                                bias=bias_sb[:], scale=SCALE2)
                        else:
                            nc.vector.tensor_copy(et[:, 0:lcnt],
                                                  st_ps[:, 0:lcnt])
                        nc.scalar.activation(
                            pt[:, js, 0, ljs:L], et[:, 0:lcnt],
                            mybir.ActivationFunctionType.Copy)
                        nc.vector.tensor_sub(pt[:, js, 1, ljs:L],
                                             et[:, 0:lcnt],
                                             pt[:, js, 0, ljs:L])
                    pt_tiles[W] = pt

                def finalize_half(ci, half, src_ap):
                    # half 1 = cols [512, 1024) (recip fires first); 0 = rest
                    if half == 1:
                        if 'rs' in parts and 'pv' in parts:
                            nc.vector.reciprocal(recip_sb[:, ci:ci + 1],
                                                 rs_ps[:, ci:ci + 1])
                        else:
                            nc.vector.memset(recip_sb[:, ci:ci + 1], 1.0)
                        o_t = outp.tile([P, D], bf16, name=f"o{ci}", tag="o")
                        o_tiles[ci] = o_t
                    else:
                        o_t = o_tiles[ci]
                    d0 = half * 512
                    nc.scalar.activation(o_t[:, d0:d0 + 512], src_ap,
                                         mybir.ActivationFunctionType.Copy,
                                         scale=recip_sb[:, ci:ci + 1])
                    nc.sync.dma_start(
                        bass.AP(outd, ci * P * D + d0, [[D, P], [1, 512]]),
                        o_t[:, d0:d0 + 512])

                rs_state = {'started': False}

                def pv_dr(pv, pt, v_w, first_w, last_w, fin_ci=None):
                    f32r = mybir.dt.float32r
                    for dh in range(2):
                        gi, glast = 0, 2 * 3 - 1
                        a = dh * 512
                        for jp2 in (0, 2):
                            for ra, rb in PRODUCTS:
                                nc.tensor.matmul(
                                    pv[dh][:],
                                    pt[:, jp2:jp2 + 2, ra, :],
                                    v_w[:, jp2:jp2 + 2, rb, a:a + 512],
                                    start=(first_w and gi == 0),
                                    stop=(last_w and gi == glast
                                          and fin_ci is None),
                                    perf_mode=DR)
                                gi += 1
                        if last_w and fin_ci is not None:
                            # fold the SBUF accumulator into this PSUM bank
                            # on the PE (f32r runs at full rate), so the
                            # scale/store reads PSUM directly with no DVE add
                            nc.tensor.matmul(
                                pv[dh][:],
                                id_sb[:],
                                acc[:, fin_ci, a:a + 512],
                                start=False, stop=True,
                                skip_group_check=True)

                def rs_dr(ci, pt, rs_last):
                    # row sums: one accumulation group spanning all windows
                    if 'rs' not in parts:
                        return
                    for gi, (jp2, role) in enumerate(
                            ((0, 0), (0, 1), (2, 0), (2, 1))):
                        nc.tensor.matmul(
                            rs_ps[:, ci:ci + 1],
                            pt[:, jp2:jp2 + 2, role, :],
                            ones_t[:, jp2:jp2 + 2, :],
                            start=(not rs_state['started']),
                            stop=(rs_last and gi == 3),
                            perf_mode=DR)
                        rs_state['started'] = True

                def pv_rs(W):
                    # windows >= 4 touch at most two l-chunks, so window
                    # pairs (4,5) and (6,7) accumulate directly in PSUM
                    ci0 = W // 2
                    pt = pt_tiles[W]
                    v_w = v_tiles[W]
                    paired = W >= 4
                    first = (not paired) or (W % 2 == 0)
                    last = (not paired) or (W % 2 == 1)
                    if 'pv' not in parts:
                        return
                    for ci in range(ci0, 4):
                        ptc = pt[:, :, :, ci * P:(ci + 1) * P]
                        rs_dr(ci, ptc, rs_last=(W == N_WIN - 1 and ci == 3))
                        if first:
                            pv = [pvps.tile([P, 512], f32,
                                            name=f"pv{W}_{ci}_{dh}", tag="pv")
                                  for dh in range(2)]
                            pv_pair[ci] = pv
                        else:
                            pv = pv_pair[ci]
                        fin = (last and W == 2 * ci + 1 and W != 0)
                        pv_dr(pv, ptc, v_w, first, last,
                              fin_ci=(ci if fin else None))
                        if not last:
                            continue
                        if W == 0:
                            for dh in range(2):
                                nc.vector.tensor_copy(
                                    acc[:, ci, dh * 512:dh * 512 + 512],
                                    pv[dh][:])
                        elif W == 2 * ci + 1 and ci == 3:
                            # last chunk: scale/store straight from PSUM in
                            # 256-col pieces; alternate DMA issue queues
                            nc.vector.reciprocal(recip_sb[:, 3:4],
                                                 rs_ps[:, 3:4])
                            o_t = outp.tile([P, D], bf16, name="o3", tag="o")
                            for i, a in enumerate(range(0, D, 512)):
                                nc.scalar.activation(
                                    o_t[:, a:a + 512],
                                    pv[a // 512][:],
                                    mybir.ActivationFunctionType.Copy,
                                    scale=recip_sb[:, 3:4])
                                eng = nc.sync if i % 2 == 0 else nc.scalar
                                eng.dma_start(
                                    bass.AP(outd, 3 * P * D + a,
                                            [[D, P], [1, 512]]),
                                    o_t[:, a:a + 512])
                        elif W == 2 * ci + 1:
                            finalize_half(ci, 1, pv[1][:])
                            finalize_half(ci, 0, pv[0][:])
                        else:
                            for dh in range(2):
                                nc.vector.tensor_add(
                                    acc[:, ci, dh * 512:dh * 512 + 512],
                                    acc[:, ci, dh * 512:dh * 512 + 512],
                                    pv[dh][:])

                # software pipeline: PE does scores(W+1) before pv(W)
                dma_win_kt(0)
                dma_win_v(0)
                dma_win_kt(1)
                dma_win_v(1)
                scores(0)
                for W in range(N_WIN):
                    if W + 2 < N_WIN:
                        dma_win_kt(W + 2)
                        dma_win_v(W + 2)
                    if W + 1 < N_WIN:
                        scores(W + 1)
                    pv_rs(W)

    nc.compile()
    return nc


def _host_inputs(x, W_query, W_key, W_value):
    f8 = ml_dtypes.float8_e4m3

    def split8(a):
        hi = a.astype(f8)
        lo = (a - hi.astype(np.float32)).astype(f8)
        return hi, lo

    def wprep(W):
        # [feat, dcol] * 32 -> [P, 8 ko, 2 role, D] e4m3 pairs
        hi, lo = split8(np.ascontiguousarray(W * 32.0, dtype=np.float32))
        out = np.empty((P, 8, 2, D), f8)
        out[:, :, 0, :] = hi.reshape(8, P, D).transpose(1, 0, 2)
        out[:, :, 1, :] = lo.reshape(8, P, D).transpose(1, 0, 2)
        return out

    def xprep(rows):
        xt = np.ascontiguousarray(x[rows].T, dtype=np.float32)  # [feat, 512]
        hi, lo = split8(xt)
        out = np.empty((P, 8, 2, L), f8)
        out[:, :, 0, :] = hi.reshape(8, P, L).transpose(1, 0, 2)
        out[:, :, 1, :] = lo.reshape(8, P, L).transpose(1, 0, 2)
        return out

    wq_n, wk_n, wv_n = wprep(W_query), wprep(W_key), wprep(W_value)

    in_maps = []
    for c in range(N_CORES):
        # mask[jp, lb]: within the 16-col band at l = 64W + 16js + lb, the
        # key j = 512W + 128js + jp is valid iff jp <= 8*lb + c  (same for
        # every window W and key block js)
        jp = np.arange(P)[:, None]
        lb = np.arange(16)[None, :]
        mask = np.where(jp <= 8 * lb + c, 0.0, BIG_NEG).astype(np.float32)
        in_maps.append({
            "wq": wq_n, "wk": wk_n, "wv": wv_n,
            "xkv": xprep(np.arange(L * c, L * (c + 1))),
            "xq": xprep(np.arange(L) * 8 + c),
            "mask": np.ascontiguousarray(mask),
            "ident": np.eye(P, dtype=np.float32),
        })
    return in_maps


def kernel(x, W_query, W_key, W_value):
    from concourse.bass_utils import run_bass_kernel_spmd

    x = np.asarray(x, dtype=np.float32)
    W_query = np.asarray(W_query, dtype=np.float32)
    W_key = np.asarray(W_key, dtype=np.float32)
    W_value = np.asarray(W_value, dtype=np.float32)

    if "nc" not in _CACHE:
        _CACHE["nc"] = _build()
    nc = _CACHE["nc"]

    in_maps = _host_inputs(x, W_query, W_key, W_value)
    res = run_bass_kernel_spmd(nc, in_maps, core_ids=list(range(N_CORES)))

    out = np.empty((S, D), dtype=np.float32)
    for c in range(N_CORES):
        # device result is 32x the true output (V was pre-scaled by 32)
        out[np.arange(L) * 8 + c] = \
            res.results[c]["out"].astype(np.float32) / 32.0
    return out


# revision 58
# speedup vs baseline: 1.0017x; 1.0017x over previous
"""Causal self-attention (single head, S=4096, D=1024) on 8 TRN2 NeuronCores.

Strategy (striped sequence-parallel + split-fp8 DoubleRow matmuls):
  - Core c owns the strided query rows {i : i mod 8 == c} (local index
    l = 0..511, global i = 8l + c) and computes K/V projections for the
    contiguous rows [512c, 512(c+1)); K^T/V are AllGathered.
  - Every matmul runs in fp8e4 (e4m3) with DoubleRow perf mode, which packs
    TWO 128-deep contraction slices per instruction at 0.5 PE cycles/row.
    Each operand is carried as an (hi, lo) e4m3 pair (lo = exact residual of
    the hi quantization); a product (A_hi+A_lo)(B_hi+B_lo) is evaluated as
    hi*hi + hi*lo + lo*hi (the lo*lo term is ~1e-3 relative and dropped).
    That is 3 slice-products per pair = 1.5 DoubleRow instructions per
    128-slice, i.e. 0.75x the PE cycles of bf16 at ~bf16 accuracy.
  - W is pre-scaled by 32 on the host (W' ~ N(0,1)) so its fp8 residual
    stays in e4m3's normal range; 1/1024 folds into the softmax scale and
    the extra 32x on V divides out on the host.
  - exp() runs with a -2.0 bias so unnormalized scores stay below e4m3's
    448 max (bias cancels in the softmax normalization).
  - Scores are computed TRANSPOSED (S^T[j, l], keys on partitions) with a
    per-js causal trim: key block js of window W only attends local queries
    l >= 64W + 16js, which makes the mask band a single [128 x 16] tile
    shared by every (W, js).
  - kT projection consumes its operands ko-pair-major so the PE starts as
    soon as the first weight pair lands (the head is DMA-paced); row sums
    accumulate in a dedicated PSUM bank as one long-lived group; quantize
    work is spread across Act/DVE/GpSimd; the PE stream is software-
    pipelined (scores of window W+1 before PV of window W).
"""

import numpy as np
import ml_dtypes

S = 4096
D = 1024
N_CORES = 8
P = 128
L = 512               # local query rows per core (striped)
N_WIN = 8
KT_ELEMS = P * 8 * 2 * L      # per-rank kT block: [dp 128][ko 8][role 2][j 512]
V_ELEMS = P * 4 * 2 * D       # per-rank v block: [jp 128][js 4][role 2][d 1024]
SCALE2 = 1.0 / (32.0 * 1024.0)   # 1/sqrt(D) / (32*32) from W pre-scaling
EXP_BIAS = -2.0
BIG_NEG = -1e30

_CACHE = {}


def _build(parts=frozenset({'sc', 'pv', 'act', 'msk', 'rs', 'dma', 'acc', 'ag'})):
    import concourse.bass as bass
    import concourse.mybir as mybir
    import concourse.tile as tile
    from concourse import bacc

    bf16 = mybir.dt.bfloat16
    f32 = mybir.dt.float32
    f8 = mybir.dt.float8e4
    DR = mybir.MatmulPerfMode.DoubleRow

    nc = bacc.Bacc("TRN2", target_bir_lowering=False, debug=False,
                   num_devices=N_CORES)

    # ---- per-core I/O (all fp8 operands are (hi, lo) e4m3 pairs) ----
    wq = nc.dram_tensor("wq", [P, 8, 2, D], f8, kind="ExternalInput")
    wk = nc.dram_tensor("wk", [P, 8, 2, D], f8, kind="ExternalInput")
    wv = nc.dram_tensor("wv", [P, 8, 2, D], f8, kind="ExternalInput")
    xkv = nc.dram_tensor("xkv", [P, 8, 2, L], f8, kind="ExternalInput")
    xq = nc.dram_tensor("xq", [P, 8, 2, L], f8, kind="ExternalInput")
    maskd = nc.dram_tensor("mask", [P, 16], f32, kind="ExternalInput")
    identd = nc.dram_tensor("ident", [P, P], mybir.dt.float32r,
                            kind="ExternalInput")
    outd = nc.dram_tensor("out", [L, D], bf16, kind="ExternalOutput")

    agin_k = nc.dram_tensor("agin_k", [1, KT_ELEMS], f8)
    agout_k = nc.dram_tensor("agout_k", [1, N_CORES * KT_ELEMS], f8,
                             addr_space="Shared")
    agin_v = nc.dram_tensor("agin_v", [1, V_ELEMS], f8)
    agout_v = nc.dram_tensor("agout_v", [1, N_CORES * V_ELEMS], f8,
                             addr_space="Shared")

    PRODUCTS = ((0, 0), (0, 1), (1, 0))   # (hi,hi), (hi,lo), (lo,hi)

    def ag(agin, agout):
        if 'ag' in parts:
            nc.gpsimd.collective_compute(
                "AllGather", mybir.AluOpType.bypass,
                replica_groups=[list(range(N_CORES))],
                ins=[agin.ap().opt()],
                outs=[agout.ap().opt()],
            )
        else:
            # Local stand-in with the same per-core traffic shape.
            n = agin.shape[1]
            for sp in range(2):
                off = sp * (n // 2)
                nc.sync.dma_start(
                    bass.AP(agout, off, [[1, 1], [1, n // 2]]),
                    bass.AP(agin, off, [[1, 1], [1, n // 2]]))

    with tile.TileContext(nc) as tc:
        with tc.tile_pool(name="wpool", bufs=12) as wpool, \
             tc.tile_pool(name="xpool", bufs=8) as xpool, \
             tc.tile_pool(name="qt", bufs=1) as qtpool, \
             tc.tile_pool(name="stage", bufs=3) as stage, \
             tc.tile_pool(name="consts", bufs=1) as consts, \
             tc.tile_pool(name="accs", bufs=1) as accs:

            # ---------------- Phase 1: projections ----------------
            # per ko-pair tiles so the PE can start on pair 0 immediately
            xkvp = [xpool.tile([P, 2, 2, L], f8, name=f"xkv{i}", tag="x")
                    for i in range(4)]
            xqp = [xpool.tile([P, 2, 2, L], f8, name=f"xq{i}", tag="x")
                   for i in range(4)]
            wkp = [wpool.tile([P, 2, 2, D], f8, name=f"wk{i}", tag="w")
                   for i in range(4)]
            wvp = [wpool.tile([P, 2, 2, D], f8, name=f"wv{i}", tag="w")
                   for i in range(4)]
            wqp = [wpool.tile([P, 2, 2, D], f8, name=f"wq{i}", tag="w")
                   for i in range(4)]
            bias_sb = consts.tile([P, 1], f32, name="bias_sb")
            id_sb = consts.tile([P, P], mybir.dt.float32r, name="id_sb")
            mask_sb = consts.tile([P, 16], f32, name="mask_sb")
            ones_t = consts.tile([P, 4, 1], f8, name="ones_t")
            nc.vector.memset(bias_sb[:], EXP_BIAS)
            nc.vector.memset(ones_t[:], 1.0)
            # DMA order follows consumption: kT path (hi roles of the first
            # pair land first so the PE starts sooner), then V, then Q
            for i in range(4):
                nc.sync.dma_start(xkvp[i][:], xkv[:, 2 * i:2 * i + 2])
                nc.sync.dma_start(wkp[i][:], wk[:, 2 * i:2 * i + 2])
            for i in range(4):
                nc.sync.dma_start(wvp[i][:], wv[:, 2 * i:2 * i + 2])
                nc.sync.dma_start(xqp[i][:], xq[:, 2 * i:2 * i + 2])
            for i in range(4):
                nc.sync.dma_start(wqp[i][:], wq[:, 2 * i:2 * i + 2])
            nc.sync.dma_start(mask_sb[:], maskd[:])
            nc.sync.dma_start(id_sb[:], identd[:])

            def quant(hi_out, lo_out, src, sub_eng=None):
                # src (f32) -> hi e4m3 + exact-residual lo e4m3
                nc.scalar.activation(hi_out, src,
                                     mybir.ActivationFunctionType.Copy)
                (sub_eng or nc.vector).tensor_sub(lo_out, src, hi_out)

            halves2 = [(0, 512), (512, 1024)]

            # ---- kT projection (dt2-major; quants drain incrementally) ----
            with tc.tile_pool(name="pps", bufs=2, space="PSUM") as pps, \
                 tc.tile_pool(name="ppsv", bufs=4, space="PSUM") as ppsv:
                for dt2 in range(4):
                    ps = pps.tile([P, 2, L], f32, name=f"kt_ps{dt2}",
                                  tag="ktps")
                    for h in range(2):
                        d0 = (dt2 * 2 + h) * P
                        first = True
                        for kp in range(4):
                            for pi, (ra, rb) in enumerate(PRODUCTS):
                                nc.tensor.matmul(
                                    ps[:, h, :],
                                    wkp[kp][:, :, ra, d0:d0 + P],
                                    xkvp[kp][:, :, rb, :],
                                    start=first,
                                    stop=(kp == 3 and pi == 2),
                                    perf_mode=DR)
                                first = False
                    st = stage.tile([P, 2, 2, L], f8, name=f"kt_st{dt2}",
                                    tag="ktst")
                    for h in range(2):
                        quant(st[:, h, 0, :], st[:, h, 1, :], ps[:, h, :])
                    # agin_k layout: [dp][ko][role][j]
                    dst = bass.AP(agin_k, dt2 * 2 * 2 * L,
                                  [[8 * 2 * L, P], [2 * L, 2], [L, 2], [1, L]])
                    nc.sync.dma_start(dst, st[:])

                # ---- AllGather K (early, overlaps V/Q projections) ----
                ag(agin_k, agout_k)

                # ---- V projection (st_i-major; wv arrives during kT) ----
                for st_i in range(4):
                    st = stage.tile([P, 2, D], f8, name=f"v_st{st_i}",
                                    tag="vst")
                    for a, b in halves2:
                        # one single-bank tile per 512-col half: slots free
                        # as soon as that half's quant drains, so the next
                        # group's matmuls never wait on a whole-tile quant
                        ps = ppsv.tile([P, 512], f32,
                                       name=f"v_ps{st_i}_{a}", tag="vps")
                        for kp in range(4):
                            for pi, (ra, rb) in enumerate(PRODUCTS):
                                nc.tensor.matmul(
                                    ps[:],
                                    xkvp[kp][:, :, ra,
                                             st_i * P:(st_i + 1) * P],
                                    wvp[kp][:, :, rb, a:b],
                                    start=(kp == 0 and pi == 0),
                                    stop=(kp == 3 and pi == 2),
                                    perf_mode=DR)
                        quant(st[:, 0, a:b], st[:, 1, a:b], ps[:])
                    # agin_v layout: [jp][js][role][d]
                    dst = bass.AP(agin_v, st_i * 2 * D,
                                  [[4 * 2 * D, P], [D, 2], [1, D]])
                    nc.sync.dma_start(dst, st[:])

                ag(agin_v, agout_v)

                # ---- Q projection (dt2-major) -> qT stays in SBUF ----
                qt8 = qtpool.tile([P, 8, 2, L], f8, name="qt8")
                for dt2 in range(4):
                    for h in range(2):
                        ps = pps.tile([P, L], f32, name=f"q_ps{dt2}_{h}",
                                      tag="ktps")
                        d0 = (dt2 * 2 + h) * P
                        first = True
                        for kp in range(4):
                            for pi, (ra, rb) in enumerate(PRODUCTS):
                                nc.tensor.matmul(
                                    ps[:],
                                    wqp[kp][:, :, ra, d0:d0 + P],
                                    xqp[kp][:, :, rb, :],
                                    start=first,
                                    stop=(kp == 3 and pi == 2),
                                    perf_mode=DR)
                                first = False
                        quant(qt8[:, 2 * dt2 + h, 0, :],
                              qt8[:, 2 * dt2 + h, 1, :], ps[:])

            # ---------------- Phase 2: attention ----------------
            # f32r so the PE identity-fold can consume it directly
            acc = accs.tile([P, 4, D], mybir.dt.float32r, name="acc")
            recip_sb = consts.tile([P, 4], f32, name="recip_sb")

            with tc.tile_pool(name="ktw", bufs=3) as ktw, \
                 tc.tile_pool(name="vw", bufs=4) as vw, \
                 tc.tile_pool(name="pt", bufs=2) as ptp, \
                 tc.tile_pool(name="et", bufs=4) as etp, \
                 tc.tile_pool(name="outp", bufs=2) as outp, \
                 tc.tile_pool(name="pvps", bufs=4, space="PSUM") as pvps, \
                 tc.tile_pool(name="rsps", bufs=1, space="PSUM") as rsps, \
                 tc.tile_pool(name="stps", bufs=3, space="PSUM") as stps:

                # every window's row-sum matmuls accumulate here as ONE
                # long-lived group in a dedicated PSUM bank
                rs_ps = rsps.tile([P, 4], f32, name="rs_ps")

                kt_tiles = {}
                v_tiles = {}
                pt_tiles = {}
                pv_pair = {}
                o_tiles = {}

                def dma_win_kt(W):
                    kt_w = ktw.tile([P, 8, 2, L], f8, name=f"kt_w{W}", tag="ktw")
                    for sp in range(2):
                        nc.sync.dma_start(
                            kt_w[:, sp * 4:(sp + 1) * 4],
                            bass.AP(agout_k,
                                    W * KT_ELEMS + sp * 4 * 2 * L,
                                    [[8 * 2 * L, P], [2 * L, 4], [L, 2],
                                     [1, L]]))
                    kt_tiles[W] = kt_w

                def dma_win_v(W):
                    v_w = vw.tile([P, 4, 2, D], f8, name=f"v_w{W}", tag="vw")
                    for sp in range(2):
                        nc.sync.dma_start(
                            v_w[:, sp * 2:(sp + 1) * 2],
                            bass.AP(agout_v,
                                    W * V_ELEMS + sp * 2 * 2 * D,
                                    [[4 * 2 * D, P], [2 * D, 2], [D, 2],
                                     [1, D]]))
                    v_tiles[W] = v_w

                def scores(W):
                    l0 = 64 * W
                    lc0 = 128 * (W // 2)
                    pt = ptp.tile([P, 4, 2, L], f8, name=f"pt{W}", tag="pt")
                    kt_w = kt_tiles[W]
                    for js in range(4):
                        ljs = l0 + 16 * js
                        if ljs != lc0:
                            # zero the strip PV touches but scores skip
                            nc.vector.memset(pt[:, js, :, lc0:ljs], 0.0)
                        lcnt = L - ljs
                        st_ps = stps.tile([P, 512], f32, name=f"st{W}_{js}",
                                          tag="st")
                        if 'sc' in parts:
                            first = True
                            for kp in range(4):
                                for pi, (ra, rb) in enumerate(PRODUCTS):
                                    nc.tensor.matmul(
                                        st_ps[:, 0:lcnt],
                                        kt_w[:, 2 * kp:2 * kp + 2, ra,
                                             js * P:(js + 1) * P],
                                        qt8[:, 2 * kp:2 * kp + 2, rb,
                                            ljs:L],
                                        start=first,
                                        stop=(kp == 3 and pi == 2),
                                        perf_mode=DR)
                                    first = False
                        else:
                            nc.vector.memset(st_ps[:, 0:lcnt], 0.1)
                        if 'msk' in parts:
                            nc.vector.tensor_add(st_ps[:, 0:16],
                                                 st_ps[:, 0:16], mask_sb[:])
                        et = etp.tile([P, 512], f32, name=f"et{W}_{js}",
                                      tag="et")
                        if 'act' in parts:
                            nc.scalar.activation(
                                et[:, 0:lcnt], st_ps[:, 0:lcnt],
                                mybir.ActivationFunctionType.Exp,
                                bias=bias_sb[:], scale=SCALE2)
                        else:
                            nc.vector.tensor_copy(et[:, 0:lcnt],
                                                  st_ps[:, 0:lcnt])
                        nc.scalar.activation(
                            pt[:, js, 0, ljs:L], et[:, 0:lcnt],
                            mybir.ActivationFunctionType.Copy)
                        nc.vector.tensor_sub(pt[:, js, 1, ljs:L],
                                             et[:, 0:lcnt],
                                             pt[:, js, 0, ljs:L])
                    pt_tiles[W] = pt

                def finalize_half(ci, half, src_ap):
                    # half 1 = cols [512, 1024) (recip fires first); 0 = rest
                    if half == 1:
                        if 'rs' in parts and 'pv' in parts:
                            nc.vector.reciprocal(recip_sb[:, ci:ci + 1],
                                                 rs_ps[:, ci:ci + 1])
                        else:
                            nc.vector.memset(recip_sb[:, ci:ci + 1], 1.0)
                        o_t = outp.tile([P, D], bf16, name=f"o{ci}", tag="o")
                        o_tiles[ci] = o_t
                    else:
                        o_t = o_tiles[ci]
                    d0 = half * 512
                    nc.scalar.activation(o_t[:, d0:d0 + 512], src_ap,
                                         mybir.ActivationFunctionType.Copy,
                                         scale=recip_sb[:, ci:ci + 1])
                    nc.sync.dma_start(
                        bass.AP(outd, ci * P * D + d0, [[D, P], [1, 512]]),
                        o_t[:, d0:d0 + 512])

                rs_state = {'started': False}

                def pv_dr(pv, pt, v_w, first_w, last_w, fin_ci=None):
                    f32r = mybir.dt.float32r
                    for dh in range(2):
                        gi, glast = 0, 2 * 3 - 1
                        a = dh * 512
                        for jp2 in (0, 2):
                            for ra, rb in PRODUCTS:
                                nc.tensor.matmul(
                                    pv[dh][:],
                                    pt[:, jp2:jp2 + 2, ra, :],
                                    v_w[:, jp2:jp2 + 2, rb, a:a + 512],
                                    start=(first_w and gi == 0),
                                    stop=(last_w and gi == glast
                                          and fin_ci is None),
                                    perf_mode=DR)
                                gi += 1
                        if last_w and fin_ci is not None:
                            # fold the SBUF accumulator into this PSUM bank
                            # on the PE (f32r runs at full rate), so the
                            # scale/store reads PSUM directly with no DVE add
                            nc.tensor.matmul(
                                pv[dh][:],
                                id_sb[:],
                                acc[:, fin_ci, a:a + 512],
                                start=False, stop=True,
                                skip_group_check=True)

                def rs_dr(ci, pt, rs_last):
                    # row sums: one accumulation group spanning all windows
                    if 'rs' not in parts:
                        return
                    for gi, (jp2, role) in enumerate(
                            ((0, 0), (0, 1), (2, 0), (2, 1))):
                        nc.tensor.matmul(
                            rs_ps[:, ci:ci + 1],
                            pt[:, jp2:jp2 + 2, role, :],
                            ones_t[:, jp2:jp2 + 2, :],
                            start=(not rs_state['started']),
                            stop=(rs_last and gi == 3),
                            perf_mode=DR)
                        rs_state['started'] = True

                def pv_rs(W):
                    # windows >= 4 touch at most two l-chunks, so window
                    # pairs (4,5) and (6,7) accumulate directly in PSUM
                    ci0 = W // 2
                    pt = pt_tiles[W]
                    v_w = v_tiles[W]
                    paired = W >= 4
                    first = (not paired) or (W % 2 == 0)
                    last = (not paired) or (W % 2 == 1)
                    if 'pv' not in parts:
                        return
                    for ci in range(ci0, 4):
                        ptc = pt[:, :, :, ci * P:(ci + 1) * P]
                        rs_dr(ci, ptc, rs_last=(W == N_WIN - 1 and ci == 3))
                        if first:
                            pv = [pvps.tile([P, 512], f32,
                                            name=f"pv{W}_{ci}_{dh}", tag="pv")
                                  for dh in range(2)]
                            pv_pair[ci] = pv
                        else:
                            pv = pv_pair[ci]
                        fin = (last and W == 2 * ci + 1 and W != 0)
                        pv_dr(pv, ptc, v_w, first, last,
                              fin_ci=(ci if fin else None))
                        if not last:
                            continue
                        if W == 0:
                            for dh in range(2):
                                nc.vector.tensor_copy(
                                    acc[:, ci, dh * 512:dh * 512 + 512],
                                    pv[dh][:])
                        elif W == 2 * ci + 1 and ci == 3:
                            # last chunk: scale/store straight from PSUM in
                            # 256-col pieces; alternate DMA issue queues
                            nc.vector.reciprocal(recip_sb[:, 3:4],
                                                 rs_ps[:, 3:4])
                            o_t = outp.tile([P, D], bf16, name="o3", tag="o")
                            for i, a in enumerate(range(0, D, 512)):
                                nc.scalar.activation(
                                    o_t[:, a:a + 512],
                                    pv[a // 512][:],
                                    mybir.ActivationFunctionType.Copy,
                                    scale=recip_sb[:, 3:4])
                                eng = nc.sync if i % 2 == 0 else nc.scalar
                                eng.dma_start(
                                    bass.AP(outd, 3 * P * D + a,
                                            [[D, P], [1, 512]]),
                                    o_t[:, a:a + 512])
                        elif W == 2 * ci + 1:
                            finalize_half(ci, 1, pv[1][:])
                            finalize_half(ci, 0, pv[0][:])
                        else:
                            for dh in range(2):
                                nc.vector.tensor_add(
                                    acc[:, ci, dh * 512:dh * 512 + 512],
                                    acc[:, ci, dh * 512:dh * 512 + 512],
                                    pv[dh][:])

                # software pipeline: PE does scores(W+1) before pv(W)
                dma_win_kt(0)
                dma_win_v(0)
                dma_win_kt(1)
                dma_win_v(1)
                scores(0)
                for W in range(N_WIN):
                    if W + 2 < N_WIN:
                        dma_win_kt(W + 2)
                        dma_win_v(W + 2)
                    if W + 1 < N_WIN:
                        scores(W + 1)
                    pv_rs(W)

    nc.compile()
    return nc


def _host_inputs(x, W_query, W_key, W_value):
    f8 = ml_dtypes.float8_e4m3

    def split8(a):
        hi = a.astype(f8)
        lo = (a - hi.astype(np.float32)).astype(f8)
        return hi, lo

    def wprep(W):
        # [feat, dcol] * 32 -> [P, 8 ko, 2 role, D] e4m3 pairs
        hi, lo = split8(np.ascontiguousarray(W * 32.0, dtype=np.float32))
        out = np.empty((P, 8, 2, D), f8)
        out[:, :, 0, :] = hi.reshape(8, P, D).transpose(1, 0, 2)
        out[:, :, 1, :] = lo.reshape(8, P, D).transpose(1, 0, 2)
        return out

    def xprep(rows):
        xt = np.ascontiguousarray(x[rows].T, dtype=np.float32)  # [feat, 512]
        hi, lo = split8(xt)
        out = np.empty((P, 8, 2, L), f8)
        out[:, :, 0, :] = hi.reshape(8, P, L).transpose(1, 0, 2)
        out[:, :, 1, :] = lo.reshape(8, P, L).transpose(1, 0, 2)
        return out

    wq_n, wk_n, wv_n = wprep(W_query), wprep(W_key), wprep(W_value)

    in_maps = []
    for c in range(N_CORES):
        # mask[jp, lb]: within the 16-col band at l = 64W + 16js + lb, the
        # key j = 512W + 128js + jp is valid iff jp <= 8*lb + c  (same for
        # every window W and key block js)
        jp = np.arange(P)[:, None]
        lb = np.arange(16)[None, :]
        mask = np.where(jp <= 8 * lb + c, 0.0, BIG_NEG).astype(np.float32)
        in_maps.append({
            "wq": wq_n, "wk": wk_n, "wv": wv_n,
            "xkv": xprep(np.arange(L * c, L * (c + 1))),
            "xq": xprep(np.arange(L) * 8 + c),
            "mask": np.ascontiguousarray(mask),
            "ident": np.eye(P, dtype=np.float32),
        })
    return in_maps


def kernel(x, W_query, W_key, W_value):
    from concourse.bass_utils import run_bass_kernel_spmd

    x = np.asarray(x, dtype=np.float32)
    W_query = np.asarray(W_query, dtype=np.float32)
    W_key = np.asarray(W_key, dtype=np.float32)
    W_value = np.asarray(W_value, dtype=np.float32)

    if "nc" not in _CACHE:
        _CACHE["nc"] = _build()
    nc = _CACHE["nc"]

    in_maps = _host_inputs(x, W_query, W_key, W_value)
    res = run_bass_kernel_spmd(nc, in_maps, core_ids=list(range(N_CORES)))

    out = np.empty((S, D), dtype=np.float32)
    for c in range(N_CORES):
        # device result is 32x the true output (V was pre-scaled by 32)
        out[np.arange(L) * 8 + c] = \
            res.results[c]["out"].astype(np.float32) / 32.0
    return out


# revision 62
# speedup vs baseline: 1.0206x; 1.0189x over previous
"""Causal self-attention (single head, S=4096, D=1024) on 8 TRN2 NeuronCores.

Strategy (striped sequence-parallel + split-fp8 DoubleRow matmuls):
  - Core c owns the strided query rows {i : i mod 8 == c} (local index
    l = 0..511, global i = 8l + c) and computes K/V projections for the
    contiguous rows [512c, 512(c+1)); K^T/V are AllGathered.
  - Every matmul runs in fp8e4 (e4m3) with DoubleRow perf mode, which packs
    TWO 128-deep contraction slices per instruction at 0.5 PE cycles/row.
    Each operand is carried as an (hi, lo) e4m3 pair (lo = exact residual of
    the hi quantization); a product (A_hi+A_lo)(B_hi+B_lo) is evaluated as
    hi*hi + hi*lo + lo*hi (the lo*lo term is ~1e-3 relative and dropped).
    That is 3 slice-products per pair = 1.5 DoubleRow instructions per
    128-slice, i.e. 0.75x the PE cycles of bf16 at ~bf16 accuracy.
  - W is pre-scaled by 32 on the host (W' ~ N(0,1)) so its fp8 residual
    stays in e4m3's normal range; 1/1024 folds into the softmax scale and
    the extra 32x on V divides out on the host.
  - exp() runs with a -2.0 bias so unnormalized scores stay below e4m3's
    448 max (bias cancels in the softmax normalization).
  - Scores are computed TRANSPOSED (S^T[j, l], keys on partitions) with a
    per-js causal trim: key block js of window W only attends local queries
    l >= 64W + 16js, which makes the mask band a single [128 x 16] tile
    shared by every (W, js).
  - kT projection consumes its operands ko-pair-major so the PE starts as
    soon as the first weight pair lands (the head is DMA-paced); row sums
    accumulate in a dedicated PSUM bank as one long-lived group; quantize
    work is spread across Act/DVE/GpSimd; the PE stream is software-
    pipelined (scores of window W+1 before PV of window W).
"""

import numpy as np
import ml_dtypes

S = 4096
D = 1024
N_CORES = 8
P = 128
L = 512               # local query rows per core (striped)
N_WIN = 8
KT_ELEMS = P * 8 * 2 * L      # per-rank kT block: [dp 128][ko 8][role 2][j 512]
V_ELEMS = P * 4 * 2 * D       # per-rank v block: [jp 128][js 4][role 2][d 1024]
SCALE2 = 1.0 / (32.0 * 1024.0)   # 1/sqrt(D) / (32*32) from W pre-scaling
EXP_BIAS = -2.0
BIG_NEG = -1e30

_CACHE = {}


def _build(parts=frozenset({'sc', 'pv', 'act', 'msk', 'rs', 'dma', 'acc', 'ag'})):
    import concourse.bass as bass
    import concourse.mybir as mybir
    import concourse.tile as tile
    from concourse import bacc

    bf16 = mybir.dt.bfloat16
    f32 = mybir.dt.float32
    f8 = mybir.dt.float8e4
    DR = mybir.MatmulPerfMode.DoubleRow

    nc = bacc.Bacc("TRN2", target_bir_lowering=False, debug=False,
                   num_devices=N_CORES)

    # ---- per-core I/O (all fp8 operands are (hi, lo) e4m3 pairs) ----
    wq = nc.dram_tensor("wq", [P, 8, 2, D], f8, kind="ExternalInput")
    wk = nc.dram_tensor("wk", [P, 8, 2, D], f8, kind="ExternalInput")
    wv = nc.dram_tensor("wv", [P, 8, 2, D], f8, kind="ExternalInput")
    xkv = nc.dram_tensor("xkv", [P, 8, 2, L], f8, kind="ExternalInput")
    xq = nc.dram_tensor("xq", [P, 8, 2, L], f8, kind="ExternalInput")
    maskd = nc.dram_tensor("mask", [P, 16], f32, kind="ExternalInput")
    identd = nc.dram_tensor("ident", [P, P], mybir.dt.float32r,
                            kind="ExternalInput")
    outd = nc.dram_tensor("out", [L, D], bf16, kind="ExternalOutput")

    agin_k = nc.dram_tensor("agin_k", [1, KT_ELEMS], f8)
    agout_k = nc.dram_tensor("agout_k", [1, N_CORES * KT_ELEMS], f8,
                             addr_space="Shared")
    agin_v = nc.dram_tensor("agin_v", [1, V_ELEMS], f8)
    agout_v = nc.dram_tensor("agout_v", [1, N_CORES * V_ELEMS], f8,
                             addr_space="Shared")

    PRODUCTS = ((0, 0), (0, 1), (1, 0))   # (hi,hi), (hi,lo), (lo,hi)

    def ag(agin, agout):
        if 'ag' in parts:
            nc.gpsimd.collective_compute(
                "AllGather", mybir.AluOpType.bypass,
                replica_groups=[list(range(N_CORES))],
                ins=[agin.ap().opt()],
                outs=[agout.ap().opt()],
            )
        else:
            # Local stand-in with the same per-core traffic shape.
            n = agin.shape[1]
            for sp in range(2):
                off = sp * (n // 2)
                nc.sync.dma_start(
                    bass.AP(agout, off, [[1, 1], [1, n // 2]]),
                    bass.AP(agin, off, [[1, 1], [1, n // 2]]))

    with tile.TileContext(nc) as tc:
        with tc.tile_pool(name="wpool", bufs=12) as wpool, \
             tc.tile_pool(name="xpool", bufs=8) as xpool, \
             tc.tile_pool(name="qt", bufs=1) as qtpool, \
             tc.tile_pool(name="stage", bufs=3) as stage, \
             tc.tile_pool(name="consts", bufs=1) as consts, \
             tc.tile_pool(name="accs", bufs=1) as accs:

            # ---------------- Phase 1: projections ----------------
            # per ko-pair tiles so the PE can start on pair 0 immediately
            xkvp = [xpool.tile([P, 2, 2, L], f8, name=f"xkv{i}", tag="x")
                    for i in range(4)]
            xqp = [xpool.tile([P, 2, 2, L], f8, name=f"xq{i}", tag="x")
                   for i in range(4)]
            wkp = [wpool.tile([P, 2, 2, D], f8, name=f"wk{i}", tag="w")
                   for i in range(4)]
            wvp = [wpool.tile([P, 2, 2, D], f8, name=f"wv{i}", tag="w")
                   for i in range(4)]
            wqp = [wpool.tile([P, 2, 2, D], f8, name=f"wq{i}", tag="w")
                   for i in range(4)]
            bias_sb = consts.tile([P, 1], f32, name="bias_sb")
            id_sb = consts.tile([P, P], mybir.dt.float32r, name="id_sb")
            mask_sb = consts.tile([P, 16], f32, name="mask_sb")
            ones_t = consts.tile([P, 4, 1], f8, name="ones_t")
            nc.vector.memset(bias_sb[:], EXP_BIAS)
            nc.vector.memset(ones_t[:], 1.0)
            # DMA order follows consumption: kT path (hi roles of the first
            # pair land first so the PE starts sooner), then V, then Q
            for i in range(4):
                nc.sync.dma_start(xkvp[i][:], xkv[:, 2 * i:2 * i + 2])
                nc.sync.dma_start(wkp[i][:], wk[:, 2 * i:2 * i + 2])
            for i in range(4):
                nc.sync.dma_start(wvp[i][:], wv[:, 2 * i:2 * i + 2])
                nc.sync.dma_start(xqp[i][:], xq[:, 2 * i:2 * i + 2])
            for i in range(4):
                nc.sync.dma_start(wqp[i][:], wq[:, 2 * i:2 * i + 2])
            nc.sync.dma_start(mask_sb[:], maskd[:])
            nc.sync.dma_start(id_sb[:], identd[:])

            def quant(hi_out, lo_out, src, sub_eng=None):
                # src (f32) -> hi e4m3 + exact-residual lo e4m3
                nc.scalar.activation(hi_out, src,
                                     mybir.ActivationFunctionType.Copy)
                (sub_eng or nc.vector).tensor_sub(lo_out, src, hi_out)

            halves2 = [(0, 512), (512, 1024)]

            # ---- kT projection (dt2-major; quants drain incrementally) ----
            with tc.tile_pool(name="pps", bufs=2, space="PSUM") as pps, \
                 tc.tile_pool(name="ppsv", bufs=4, space="PSUM") as ppsv:
                for dt2 in range(4):
                    ps = pps.tile([P, 2, L], f32, name=f"kt_ps{dt2}",
                                  tag="ktps")
                    for h in range(2):
                        d0 = (dt2 * 2 + h) * P
                        first = True
                        for kp in range(4):
                            for pi, (ra, rb) in enumerate(PRODUCTS):
                                nc.tensor.matmul(
                                    ps[:, h, :],
                                    wkp[kp][:, :, ra, d0:d0 + P],
                                    xkvp[kp][:, :, rb, :],
                                    start=first,
                                    stop=(kp == 3 and pi == 2),
                                    perf_mode=DR)
                                first = False
                    st = stage.tile([P, 2, 2, L], f8, name=f"kt_st{dt2}",
                                    tag="ktst")
                    for h in range(2):
                        quant(st[:, h, 0, :], st[:, h, 1, :], ps[:, h, :])
                    # agin_k layout: [dp][ko][role][j]
                    dst = bass.AP(agin_k, dt2 * 2 * 2 * L,
                                  [[8 * 2 * L, P], [2 * L, 2], [L, 2], [1, L]])
                    nc.sync.dma_start(dst, st[:])

                # ---- AllGather K (early, overlaps V/Q projections) ----
                ag(agin_k, agout_k)

                # ---- V projection (st_i-major; wv arrives during kT) ----
                for st_i in range(4):
                    st = stage.tile([P, 2, D], f8, name=f"v_st{st_i}",
                                    tag="vst")
                    for a, b in halves2:
                        # one single-bank tile per 512-col half: slots free
                        # as soon as that half's quant drains, so the next
                        # group's matmuls never wait on a whole-tile quant
                        ps = ppsv.tile([P, 512], f32,
                                       name=f"v_ps{st_i}_{a}", tag="vps")
                        for kp in range(4):
                            for pi, (ra, rb) in enumerate(PRODUCTS):
                                nc.tensor.matmul(
                                    ps[:],
                                    xkvp[kp][:, :, ra,
                                             st_i * P:(st_i + 1) * P],
                                    wvp[kp][:, :, rb, a:b],
                                    start=(kp == 0 and pi == 0),
                                    stop=(kp == 3 and pi == 2),
                                    perf_mode=DR)
                        quant(st[:, 0, a:b], st[:, 1, a:b], ps[:])
                    # agin_v layout: [jp][js][role][d]
                    dst = bass.AP(agin_v, st_i * 2 * D,
                                  [[4 * 2 * D, P], [D, 2], [1, D]])
                    nc.sync.dma_start(dst, st[:])

                ag(agin_v, agout_v)

                # ---- Q projection (dt2-major) -> qT stays in SBUF ----
                qt8 = qtpool.tile([P, 8, 2, L], f8, name="qt8")
                for dt2 in range(4):
                    for h in range(2):
                        ps = pps.tile([P, L], f32, name=f"q_ps{dt2}_{h}",
                                      tag="ktps")
                        d0 = (dt2 * 2 + h) * P
                        first = True
                        for kp in range(4):
                            for pi, (ra, rb) in enumerate(PRODUCTS):
                                nc.tensor.matmul(
                                    ps[:],
                                    wqp[kp][:, :, ra, d0:d0 + P],
                                    xqp[kp][:, :, rb, :],
                                    start=first,
                                    stop=(kp == 3 and pi == 2),
                                    perf_mode=DR)
                                first = False
                        quant(qt8[:, 2 * dt2 + h, 0, :],
                              qt8[:, 2 * dt2 + h, 1, :], ps[:])

            # ---------------- Phase 2: attention ----------------
            # f32r so the PE identity-fold can consume it directly
            acc = accs.tile([P, 4, D], mybir.dt.float32r, name="acc")
            recip_sb = consts.tile([P, 4], f32, name="recip_sb")

            with tc.tile_pool(name="ktw", bufs=3) as ktw, \
                 tc.tile_pool(name="vw", bufs=4) as vw, \
                 tc.tile_pool(name="pt", bufs=2) as ptp, \
                 tc.tile_pool(name="et", bufs=4) as etp, \
                 tc.tile_pool(name="outp", bufs=2) as outp, \
                 tc.tile_pool(name="pvps", bufs=4, space="PSUM") as pvps, \
                 tc.tile_pool(name="rsps", bufs=1, space="PSUM") as rsps, \
                 tc.tile_pool(name="stps", bufs=3, space="PSUM") as stps:

                # every window's row-sum matmuls accumulate here as ONE
                # long-lived group in a dedicated PSUM bank
                rs_ps = rsps.tile([P, 4], f32, name="rs_ps")

                kt_tiles = {}
                v_tiles = {}
                pt_tiles = {}
                pv_pair = {}
                o_tiles = {}

                def dma_win_kt(W):
                    kt_w = ktw.tile([P, 8, 2, L], f8, name=f"kt_w{W}", tag="ktw")
                    for sp in range(2):
                        nc.sync.dma_start(
                            kt_w[:, sp * 4:(sp + 1) * 4],
                            bass.AP(agout_k,
                                    W * KT_ELEMS + sp * 4 * 2 * L,
                                    [[8 * 2 * L, P], [2 * L, 4], [L, 2],
                                     [1, L]]))
                    kt_tiles[W] = kt_w

                def dma_win_v(W):
                    v_w = vw.tile([P, 4, 2, D], f8, name=f"v_w{W}", tag="vw")
                    for sp in range(2):
                        nc.sync.dma_start(
                            v_w[:, sp * 2:(sp + 1) * 2],
                            bass.AP(agout_v,
                                    W * V_ELEMS + sp * 2 * 2 * D,
                                    [[4 * 2 * D, P], [2 * D, 2], [D, 2],
                                     [1, D]]))
                    v_tiles[W] = v_w

                def scores(W):
                    l0 = 64 * W
                    lc0 = 128 * (W // 2)
                    pt = ptp.tile([P, 4, 2, L], f8, name=f"pt{W}", tag="pt")
                    kt_w = kt_tiles[W]
                    for js in range(4):
                        ljs = l0 + 16 * js
                        if ljs != lc0:
                            # zero the strip PV touches but scores skip
                            nc.vector.memset(pt[:, js, :, lc0:ljs], 0.0)
                        lcnt = L - ljs
                        st_ps = stps.tile([P, 512], f32, name=f"st{W}_{js}",
                                          tag="st")
                        if 'sc' in parts:
                            first = True
                            for kp in range(4):
                                for pi, (ra, rb) in enumerate(PRODUCTS):
                                    nc.tensor.matmul(
                                        st_ps[:, 0:lcnt],
                                        kt_w[:, 2 * kp:2 * kp + 2, ra,
                                             js * P:(js + 1) * P],
                                        qt8[:, 2 * kp:2 * kp + 2, rb,
                                            ljs:L],
                                        start=first,
                                        stop=(kp == 3 and pi == 2),
                                        perf_mode=DR)
                                    first = False
                        else:
                            nc.vector.memset(st_ps[:, 0:lcnt], 0.1)
                        if 'msk' in parts:
                            nc.vector.tensor_add(st_ps[:, 0:16],
                                                 st_ps[:, 0:16], mask_sb[:])
                        et = etp.tile([P, 512], f32, name=f"et{W}_{js}",
                                      tag="et")
                        if 'act' in parts:
                            nc.scalar.activation(
                                et[:, 0:lcnt], st_ps[:, 0:lcnt],
                                mybir.ActivationFunctionType.Exp,
                                bias=bias_sb[:], scale=SCALE2)
                        else:
                            nc.vector.tensor_copy(et[:, 0:lcnt],
                                                  st_ps[:, 0:lcnt])
                        nc.scalar.activation(
                            pt[:, js, 0, ljs:L], et[:, 0:lcnt],
                            mybir.ActivationFunctionType.Copy)
                        nc.vector.tensor_sub(pt[:, js, 1, ljs:L],
                                             et[:, 0:lcnt],
                                             pt[:, js, 0, ljs:L])
                    pt_tiles[W] = pt

                def finalize_half(ci, half, src_ap):
                    # half 1 = cols [512, 1024) (recip fires first); 0 = rest
                    if half == 1:
                        if 'rs' in parts and 'pv' in parts:
                            nc.vector.reciprocal(recip_sb[:, ci:ci + 1],
                                                 rs_ps[:, ci:ci + 1])
                        else:
                            nc.vector.memset(recip_sb[:, ci:ci + 1], 1.0)
                        o_t = outp.tile([P, D], bf16, name=f"o{ci}", tag="o")
                        o_tiles[ci] = o_t
                    else:
                        o_t = o_tiles[ci]
                    d0 = half * 512
                    nc.scalar.activation(o_t[:, d0:d0 + 512], src_ap,
                                         mybir.ActivationFunctionType.Copy,
                                         scale=recip_sb[:, ci:ci + 1])
                    nc.sync.dma_start(
                        bass.AP(outd, ci * P * D + d0, [[D, P], [1, 512]]),
                        o_t[:, d0:d0 + 512])

                rs_state = {'started': False}

                def pv_dr(pv, pt, v_w, first_w, last_w, fin_ci=None):
                    f32r = mybir.dt.float32r
                    for dh in range(2):
                        gi, glast = 0, 2 * 3 - 1
                        a = dh * 512
                        for jp2 in (0, 2):
                            for ra, rb in PRODUCTS:
                                nc.tensor.matmul(
                                    pv[dh][:],
                                    pt[:, jp2:jp2 + 2, ra, :],
                                    v_w[:, jp2:jp2 + 2, rb, a:a + 512],
                                    start=(first_w and gi == 0),
                                    stop=(last_w and gi == glast
                                          and fin_ci is None),
                                    perf_mode=DR)
                                gi += 1
                        if last_w and fin_ci is not None:
                            # fold the SBUF accumulator into this PSUM bank
                            # on the PE (f32r runs at full rate), so the
                            # scale/store reads PSUM directly with no DVE add
                            nc.tensor.matmul(
                                pv[dh][:],
                                id_sb[:],
                                acc[:, fin_ci, a:a + 512],
                                start=False, stop=True,
                                skip_group_check=True)

                def rs_dr(ci, pt, rs_last):
                    # row sums: one accumulation group spanning all windows
                    if 'rs' not in parts:
                        return
                    for gi, (jp2, role) in enumerate(
                            ((0, 0), (0, 1), (2, 0), (2, 1))):
                        nc.tensor.matmul(
                            rs_ps[:, ci:ci + 1],
                            pt[:, jp2:jp2 + 2, role, :],
                            ones_t[:, jp2:jp2 + 2, :],
                            start=(not rs_state['started']),
                            stop=(rs_last and gi == 3),
                            perf_mode=DR)
                        rs_state['started'] = True

                def pv_rs(W):
                    # windows >= 4 touch at most two l-chunks, so window
                    # pairs (4,5) and (6,7) accumulate directly in PSUM
                    ci0 = W // 2
                    pt = pt_tiles[W]
                    v_w = v_tiles[W]
                    paired = W >= 4
                    first = (not paired) or (W % 2 == 0)
                    last = (not paired) or (W % 2 == 1)
                    if 'pv' not in parts:
                        return
                    for ci in range(ci0, 4):
                        ptc = pt[:, :, :, ci * P:(ci + 1) * P]
                        rs_dr(ci, ptc, rs_last=(W == N_WIN - 1 and ci == 3))
                        if first:
                            pv = [pvps.tile([P, 512], f32,
                                            name=f"pv{W}_{ci}_{dh}", tag="pv")
                                  for dh in range(2)]
                            pv_pair[ci] = pv
                        else:
                            pv = pv_pair[ci]
                        fin = (last and W == 2 * ci + 1 and W != 0)
                        pv_dr(pv, ptc, v_w, first, last,
                              fin_ci=(ci if fin else None))
                        if not last:
                            continue
                        if W == 0:
                            for dh in range(2):
                                nc.vector.tensor_copy(
                                    acc[:, ci, dh * 512:dh * 512 + 512],
                                    pv[dh][:])
                        elif W == 2 * ci + 1 and ci == 3:
                            # last chunk: scale/store straight from PSUM in
                            # 256-col pieces; alternate DMA issue queues
                            nc.vector.reciprocal(recip_sb[:, 3:4],
                                                 rs_ps[:, 3:4])
                            o_t = outp.tile([P, D], bf16, name="o3", tag="o")
                            for i, a in enumerate(range(0, D, 512)):
                                nc.scalar.activation(
                                    o_t[:, a:a + 512],
                                    pv[a // 512][:],
                                    mybir.ActivationFunctionType.Copy,
                                    scale=recip_sb[:, 3:4])
                                eng = nc.sync if i % 2 == 0 else nc.scalar
                                eng.dma_start(
                                    bass.AP(outd, 3 * P * D + a,
                                            [[D, P], [1, 512]]),
                                    o_t[:, a:a + 512])
                        elif W == 2 * ci + 1:
                            finalize_half(ci, 1, pv[1][:])
                            finalize_half(ci, 0, pv[0][:])
                        else:
                            for dh in range(2):
                                nc.vector.tensor_add(
                                    acc[:, ci, dh * 512:dh * 512 + 512],
                                    acc[:, ci, dh * 512:dh * 512 + 512],
                                    pv[dh][:])

                # software pipeline: PE does scores(W+1) before pv(W)
                dma_win_kt(0)
                dma_win_v(0)
                dma_win_kt(1)
                dma_win_v(1)
                scores(0)
                for W in range(N_WIN):
                    if W + 2 < N_WIN:
                        dma_win_kt(W + 2)
                        dma_win_v(W + 2)
                    if W + 1 < N_WIN:
                        scores(W + 1)
                    pv_rs(W)

    nc.compile()
    return nc


def _host_inputs(x, W_query, W_key, W_value):
    f8 = ml_dtypes.float8_e4m3

    def split8(a):
        hi = a.astype(f8)
        lo = (a - hi.astype(np.float32)).astype(f8)
        return hi, lo

    def wprep(W):
        # [feat, dcol] * 32 -> [P, 8 ko, 2 role, D] e4m3 pairs
        hi, lo = split8(np.ascontiguousarray(W * 32.0, dtype=np.float32))
        out = np.empty((P, 8, 2, D), f8)
        out[:, :, 0, :] = hi.reshape(8, P, D).transpose(1, 0, 2)
        out[:, :, 1, :] = lo.reshape(8, P, D).transpose(1, 0, 2)
        return out

    def xprep(rows):
        xt = np.ascontiguousarray(x[rows].T, dtype=np.float32)  # [feat, 512]
        hi, lo = split8(xt)
        out = np.empty((P, 8, 2, L), f8)
        out[:, :, 0, :] = hi.reshape(8, P, L).transpose(1, 0, 2)
        out[:, :, 1, :] = lo.reshape(8, P, L).transpose(1, 0, 2)
        return out

    wq_n, wk_n, wv_n = wprep(W_query), wprep(W_key), wprep(W_value)

    in_maps = []
    for c in range(N_CORES):
        # mask[jp, lb]: within the 16-col band at l = 64W + 16js + lb, the
        # key j = 512W + 128js + jp is valid iff jp <= 8*lb + c  (same for
        # every window W and key block js)
        jp = np.arange(P)[:, None]
        lb = np.arange(16)[None, :]
        mask = np.where(jp <= 8 * lb + c, 0.0, BIG_NEG).astype(np.float32)
        in_maps.append({
            "wq": wq_n, "wk": wk_n, "wv": wv_n,
            "xkv": xprep(np.arange(L * c, L * (c + 1))),
            "xq": xprep(np.arange(L) * 8 + c),
            "mask": np.ascontiguousarray(mask),
            "ident": np.eye(P, dtype=np.float32),
        })
    return in_maps


def kernel(x, W_query, W_key, W_value):
    from concourse.bass_utils import run_bass_kernel_spmd

    x = np.asarray(x, dtype=np.float32)
    W_query = np.asarray(W_query, dtype=np.float32)
    W_key = np.asarray(W_key, dtype=np.float32)
    W_value = np.asarray(W_value, dtype=np.float32)

    if "nc" not in _CACHE:
        _CACHE["nc"] = _build()
    nc = _CACHE["nc"]

    in_maps = _host_inputs(x, W_query, W_key, W_value)
    res = run_bass_kernel_spmd(nc, in_maps, core_ids=list(range(N_CORES)))

    out = np.empty((S, D), dtype=np.float32)
    for c in range(N_CORES):
        # device result is 32x the true output (V was pre-scaled by 32)
        out[np.arange(L) * 8 + c] = \
            res.results[c]["out"].astype(np.float32) / 32.0
    return out


# revision 64
# speedup vs baseline: 1.0241x; 1.0034x over previous
"""Causal self-attention (single head, S=4096, D=1024) on 8 TRN2 NeuronCores.

Strategy (striped sequence-parallel + split-fp8 DoubleRow matmuls):
  - Core c owns the strided query rows {i : i mod 8 == c} (local index
    l = 0..511, global i = 8l + c) and computes K/V projections for the
    contiguous rows [512c, 512(c+1)); K^T/V are AllGathered.
  - Every matmul runs in fp8e4 (e4m3) with DoubleRow perf mode, which packs
    TWO 128-deep contraction slices per instruction at 0.5 PE cycles/row.
    Each operand is carried as an (hi, lo) e4m3 pair (lo = exact residual of
    the hi quantization); a product (A_hi+A_lo)(B_hi+B_lo) is evaluated as
    hi*hi + hi*lo + lo*hi (the lo*lo term is ~1e-3 relative and dropped).
    That is 3 slice-products per pair = 1.5 DoubleRow instructions per
    128-slice, i.e. 0.75x the PE cycles of bf16 at ~bf16 accuracy.
  - W is pre-scaled by 32 on the host (W' ~ N(0,1)) so its fp8 residual
    stays in e4m3's normal range; 1/1024 folds into the softmax scale and
    the extra 32x on V divides out on the host.
  - exp() runs with a -2.0 bias so unnormalized scores stay below e4m3's
    448 max (bias cancels in the softmax normalization).
  - Scores are computed TRANSPOSED (S^T[j, l], keys on partitions) with a
    per-js causal trim: key block js of window W only attends local queries
    l >= 64W + 16js, which makes the mask band a single [128 x 16] tile
    shared by every (W, js).
  - kT projection consumes its operands ko-pair-major so the PE starts as
    soon as the first weight pair lands (the head is DMA-paced); row sums
    accumulate in a dedicated PSUM bank as one long-lived group; quantize
    work is spread across Act/DVE/GpSimd; the PE stream is software-
    pipelined (scores of window W+1 before PV of window W).
"""

import numpy as np
import ml_dtypes

S = 4096
D = 1024
N_CORES = 8
P = 128
L = 512               # local query rows per core (striped)
N_WIN = 8
KT_ELEMS = P * 8 * 2 * L      # per-rank kT block: [dp 128][ko 8][role 2][j 512]
V_ELEMS = P * 4 * 2 * D       # per-rank v block: [jp 128][js 4][role 2][d 1024]
SCALE2 = 1.0 / (32.0 * 1024.0)   # 1/sqrt(D) / (32*32) from W pre-scaling
EXP_BIAS = -2.0
BIG_NEG = -1e30

_CACHE = {}


def _build(parts=frozenset({'sc', 'pv', 'act', 'msk', 'rs', 'dma', 'acc', 'ag'})):
    import concourse.bass as bass
    import concourse.mybir as mybir
    import concourse.tile as tile
    from concourse import bacc

    bf16 = mybir.dt.bfloat16
    f32 = mybir.dt.float32
    f8 = mybir.dt.float8e4
    DR = mybir.MatmulPerfMode.DoubleRow

    nc = bacc.Bacc("TRN2", target_bir_lowering=False, debug=False,
                   num_devices=N_CORES)

    # ---- per-core I/O (all fp8 operands are (hi, lo) e4m3 pairs) ----
    wq = nc.dram_tensor("wq", [P, 8, 2, D], f8, kind="ExternalInput")
    wk = nc.dram_tensor("wk", [P, 8, 2, D], f8, kind="ExternalInput")
    wv = nc.dram_tensor("wv", [P, 8, 2, D], f8, kind="ExternalInput")
    xkv = nc.dram_tensor("xkv", [P, 8, 2, L], f8, kind="ExternalInput")
    xq = nc.dram_tensor("xq", [P, 8, 2, L], f8, kind="ExternalInput")
    maskd = nc.dram_tensor("mask", [P, 16], f32, kind="ExternalInput")
    identd = nc.dram_tensor("ident", [P, P], mybir.dt.float32r,
                            kind="ExternalInput")
    outd = nc.dram_tensor("out", [L, D], bf16, kind="ExternalOutput")

    agin_k = nc.dram_tensor("agin_k", [1, KT_ELEMS], f8)
    agout_k = nc.dram_tensor("agout_k", [1, N_CORES * KT_ELEMS], f8,
                             addr_space="Shared")
    agin_v = nc.dram_tensor("agin_v", [1, V_ELEMS], f8)
    agout_v = nc.dram_tensor("agout_v", [1, N_CORES * V_ELEMS], f8,
                             addr_space="Shared")

    PRODUCTS = ((0, 0), (0, 1), (1, 0))   # (hi,hi), (hi,lo), (lo,hi)

    def ag(agin, agout):
        if 'ag' in parts:
            nc.gpsimd.collective_compute(
                "AllGather", mybir.AluOpType.bypass,
                replica_groups=[list(range(N_CORES))],
                ins=[agin.ap().opt()],
                outs=[agout.ap().opt()],
            )
        else:
            # Local stand-in with the same per-core traffic shape.
            n = agin.shape[1]
            for sp in range(2):
                off = sp * (n // 2)
                nc.sync.dma_start(
                    bass.AP(agout, off, [[1, 1], [1, n // 2]]),
                    bass.AP(agin, off, [[1, 1], [1, n // 2]]))

    with tile.TileContext(nc) as tc:
        with tc.tile_pool(name="wpool", bufs=12) as wpool, \
             tc.tile_pool(name="xpool", bufs=8) as xpool, \
             tc.tile_pool(name="qt", bufs=1) as qtpool, \
             tc.tile_pool(name="stage", bufs=3) as stage, \
             tc.tile_pool(name="consts", bufs=1) as consts, \
             tc.tile_pool(name="accs", bufs=1) as accs:

            # ---------------- Phase 1: projections ----------------
            # per ko-pair tiles so the PE can start on pair 0 immediately
            xkvp = [xpool.tile([P, 2, 2, L], f8, name=f"xkv{i}", tag="x")
                    for i in range(4)]
            xqp = [xpool.tile([P, 2, 2, L], f8, name=f"xq{i}", tag="x")
                   for i in range(4)]
            wkp = [wpool.tile([P, 2, 2, D], f8, name=f"wk{i}", tag="w")
                   for i in range(4)]
            wvp = [wpool.tile([P, 2, 2, D], f8, name=f"wv{i}", tag="w")
                   for i in range(4)]
            wqp = [wpool.tile([P, 2, 2, D], f8, name=f"wq{i}", tag="w")
                   for i in range(4)]
            bias_sb = consts.tile([P, 1], f32, name="bias_sb")
            id_sb = consts.tile([P, P], mybir.dt.float32r, name="id_sb")
            mask_sb = consts.tile([P, 16], f32, name="mask_sb")
            ones_t = consts.tile([P, 4, 1], f8, name="ones_t")
            nc.vector.memset(bias_sb[:], EXP_BIAS)
            nc.vector.memset(ones_t[:], 1.0)
            # DMA order follows consumption: kT path (hi roles of the first
            # pair land first so the PE starts sooner), then V, then Q
            for i in range(4):
                nc.sync.dma_start(xkvp[i][:], xkv[:, 2 * i:2 * i + 2])
                nc.sync.dma_start(wkp[i][:], wk[:, 2 * i:2 * i + 2])
            for i in range(4):
                nc.sync.dma_start(wvp[i][:], wv[:, 2 * i:2 * i + 2])
                nc.sync.dma_start(xqp[i][:], xq[:, 2 * i:2 * i + 2])
            for i in range(4):
                nc.sync.dma_start(wqp[i][:], wq[:, 2 * i:2 * i + 2])
            nc.sync.dma_start(mask_sb[:], maskd[:])
            nc.sync.dma_start(id_sb[:], identd[:])

            def quant(hi_out, lo_out, src, sub_eng=None):
                # src (f32) -> hi e4m3 + exact-residual lo e4m3
                nc.scalar.activation(hi_out, src,
                                     mybir.ActivationFunctionType.Copy)
                (sub_eng or nc.vector).tensor_sub(lo_out, src, hi_out)

            halves2 = [(0, 512), (512, 1024)]

            # ---- kT projection (dt2-major; quants drain incrementally) ----
            with tc.tile_pool(name="pps", bufs=2, space="PSUM") as pps, \
                 tc.tile_pool(name="ppsv", bufs=4, space="PSUM") as ppsv:
                for dt2 in range(4):
                    ps = pps.tile([P, 2, L], f32, name=f"kt_ps{dt2}",
                                  tag="ktps")
                    for h in range(2):
                        d0 = (dt2 * 2 + h) * P
                        first = True
                        for kp in range(4):
                            for pi, (ra, rb) in enumerate(PRODUCTS):
                                nc.tensor.matmul(
                                    ps[:, h, :],
                                    wkp[kp][:, :, ra, d0:d0 + P],
                                    xkvp[kp][:, :, rb, :],
                                    start=first,
                                    stop=(kp == 3 and pi == 2),
                                    perf_mode=DR)
                                first = False
                    st = stage.tile([P, 2, 2, L], f8, name=f"kt_st{dt2}",
                                    tag="ktst")
                    for h in range(2):
                        quant(st[:, h, 0, :], st[:, h, 1, :], ps[:, h, :])
                    # agin_k layout: [dp][ko][role][j]
                    dst = bass.AP(agin_k, dt2 * 2 * 2 * L,
                                  [[8 * 2 * L, P], [2 * L, 2], [L, 2], [1, L]])
                    nc.sync.dma_start(dst, st[:])

                # ---- AllGather K (early, overlaps V/Q projections) ----
                ag(agin_k, agout_k)

                # ---- V projection (st_i-major; wv arrives during kT) ----
                for st_i in range(4):
                    st = stage.tile([P, 2, D], f8, name=f"v_st{st_i}",
                                    tag="vst")
                    for a, b in halves2:
                        # one single-bank tile per 512-col half: slots free
                        # as soon as that half's quant drains, so the next
                        # group's matmuls never wait on a whole-tile quant
                        ps = ppsv.tile([P, 512], f32,
                                       name=f"v_ps{st_i}_{a}", tag="vps")
                        for kp in range(4):
                            for pi, (ra, rb) in enumerate(PRODUCTS):
                                nc.tensor.matmul(
                                    ps[:],
                                    xkvp[kp][:, :, ra,
                                             st_i * P:(st_i + 1) * P],
                                    wvp[kp][:, :, rb, a:b],
                                    start=(kp == 0 and pi == 0),
                                    stop=(kp == 3 and pi == 2),
                                    perf_mode=DR)
                        quant(st[:, 0, a:b], st[:, 1, a:b], ps[:])
                    # agin_v layout: [jp][js][role][d]
                    dst = bass.AP(agin_v, st_i * 2 * D,
                                  [[4 * 2 * D, P], [D, 2], [1, D]])
                    nc.sync.dma_start(dst, st[:])

                ag(agin_v, agout_v)

                # ---- Q projection (dt2-major) -> qT stays in SBUF ----
                qt8 = qtpool.tile([P, 8, 2, L], f8, name="qt8")
                for dt2 in range(4):
                    for h in range(2):
                        ps = pps.tile([P, L], f32, name=f"q_ps{dt2}_{h}",
                                      tag="ktps")
                        d0 = (dt2 * 2 + h) * P
                        first = True
                        for kp in range(4):
                            for pi, (ra, rb) in enumerate(PRODUCTS):
                                nc.tensor.matmul(
                                    ps[:],
                                    wqp[kp][:, :, ra, d0:d0 + P],
                                    xqp[kp][:, :, rb, :],
                                    start=first,
                                    stop=(kp == 3 and pi == 2),
                                    perf_mode=DR)
                                first = False
                        quant(qt8[:, 2 * dt2 + h, 0, :],
                              qt8[:, 2 * dt2 + h, 1, :], ps[:])

            # ---------------- Phase 2: attention ----------------
            # f32r so the PE identity-fold can consume it directly
            acc = accs.tile([P, 4, D], mybir.dt.float32r, name="acc")
            recip_sb = consts.tile([P, 4], f32, name="recip_sb")

            with tc.tile_pool(name="ktw", bufs=3) as ktw, \
                 tc.tile_pool(name="vw", bufs=4) as vw, \
                 tc.tile_pool(name="pt", bufs=2) as ptp, \
                 tc.tile_pool(name="et", bufs=4) as etp, \
                 tc.tile_pool(name="outp", bufs=2) as outp, \
                 tc.tile_pool(name="pvps", bufs=4, space="PSUM") as pvps, \
                 tc.tile_pool(name="rsps", bufs=1, space="PSUM") as rsps, \
                 tc.tile_pool(name="stps", bufs=3, space="PSUM") as stps:

                # every window's row-sum matmuls accumulate here as ONE
                # long-lived group in a dedicated PSUM bank
                rs_ps = rsps.tile([P, 4], f32, name="rs_ps")

                kt_tiles = {}
                v_tiles = {}
                pt_tiles = {}
                pv_pair = {}
                o_tiles = {}

                def dma_win_kt(W):
                    kt_w = ktw.tile([P, 8, 2, L], f8, name=f"kt_w{W}", tag="ktw")
                    for sp in range(2):
                        nc.sync.dma_start(
                            kt_w[:, sp * 4:(sp + 1) * 4],
                            bass.AP(agout_k,
                                    W * KT_ELEMS + sp * 4 * 2 * L,
                                    [[8 * 2 * L, P], [2 * L, 4], [L, 2],
                                     [1, L]]))
                    kt_tiles[W] = kt_w

                def dma_win_v(W):
                    v_w = vw.tile([P, 4, 2, D], f8, name=f"v_w{W}", tag="vw")
                    for sp in range(2):
                        nc.sync.dma_start(
                            v_w[:, sp * 2:(sp + 1) * 2],
                            bass.AP(agout_v,
                                    W * V_ELEMS + sp * 2 * 2 * D,
                                    [[4 * 2 * D, P], [2 * D, 2], [D, 2],
                                     [1, D]]))
                    v_tiles[W] = v_w

                def scores(W):
                    l0 = 64 * W
                    lc0 = 128 * (W // 2)
                    pt = ptp.tile([P, 4, 2, L], f8, name=f"pt{W}", tag="pt")
                    kt_w = kt_tiles[W]
                    for js in range(4):
                        ljs = l0 + 16 * js
                        if ljs != lc0:
                            # zero the strip PV touches but scores skip
                            nc.vector.memset(pt[:, js, :, lc0:ljs], 0.0)
                        lcnt = L - ljs
                        st_ps = stps.tile([P, 512], f32, name=f"st{W}_{js}",
                                          tag="st")
                        if 'sc' in parts:
                            first = True
                            for kp in range(4):
                                for pi, (ra, rb) in enumerate(PRODUCTS):
                                    nc.tensor.matmul(
                                        st_ps[:, 0:lcnt],
                                        kt_w[:, 2 * kp:2 * kp + 2, ra,
                                             js * P:(js + 1) * P],
                                        qt8[:, 2 * kp:2 * kp + 2, rb,
                                            ljs:L],
                                        start=first,
                                        stop=(kp == 3 and pi == 2),
                                        perf_mode=DR)
                                    first = False
                        else:
                            nc.vector.memset(st_ps[:, 0:lcnt], 0.1)
                        if 'msk' in parts:
                            nc.vector.tensor_add(st_ps[:, 0:16],
                                                 st_ps[:, 0:16], mask_sb[:])
                        et = etp.tile([P, 512], f32, name=f"et{W}_{js}",
                                      tag="et")
                        if 'act' in parts:
                            nc.scalar.activation(
                                et[:, 0:lcnt], st_ps[:, 0:lcnt],
                                mybir.ActivationFunctionType.Exp,
                                bias=bias_sb[:], scale=SCALE2)
                        else:
                            nc.vector.tensor_copy(et[:, 0:lcnt],
                                                  st_ps[:, 0:lcnt])
                        nc.scalar.activation(
                            pt[:, js, 0, ljs:L], et[:, 0:lcnt],
                            mybir.ActivationFunctionType.Copy)
                        nc.vector.tensor_sub(pt[:, js, 1, ljs:L],
                                             et[:, 0:lcnt],
                                             pt[:, js, 0, ljs:L])
                    pt_tiles[W] = pt

                def finalize_half(ci, half, src_ap):
                    # half 1 = cols [512, 1024) (recip fires first); 0 = rest
                    if half == 1:
                        if 'rs' in parts and 'pv' in parts:
                            nc.vector.reciprocal(recip_sb[:, ci:ci + 1],
                                                 rs_ps[:, ci:ci + 1])
                        else:
                            nc.vector.memset(recip_sb[:, ci:ci + 1], 1.0)
                        o_t = outp.tile([P, D], bf16, name=f"o{ci}", tag="o")
                        o_tiles[ci] = o_t
                    else:
                        o_t = o_tiles[ci]
                    d0 = half * 512
                    nc.scalar.activation(o_t[:, d0:d0 + 512], src_ap,
                                         mybir.ActivationFunctionType.Copy,
                                         scale=recip_sb[:, ci:ci + 1])
                    nc.sync.dma_start(
                        bass.AP(outd, ci * P * D + d0, [[D, P], [1, 512]]),
                        o_t[:, d0:d0 + 512])

                rs_state = {'started': False}

                def pv_dr(pv, pt, v_w, first_w, last_w, fin_ci=None):
                    f32r = mybir.dt.float32r
                    for dh in range(2):
                        gi, glast = 0, 2 * 3 - 1
                        a = dh * 512
                        for jp2 in (0, 2):
                            for ra, rb in PRODUCTS:
                                nc.tensor.matmul(
                                    pv[dh][:],
                                    pt[:, jp2:jp2 + 2, ra, :],
                                    v_w[:, jp2:jp2 + 2, rb, a:a + 512],
                                    start=(first_w and gi == 0),
                                    stop=(last_w and gi == glast
                                          and fin_ci is None),
                                    perf_mode=DR)
                                gi += 1
                        if last_w and fin_ci is not None:
                            # fold the SBUF accumulator into this PSUM bank
                            # on the PE (f32r runs at full rate), so the
                            # scale/store reads PSUM directly with no DVE add
                            nc.tensor.matmul(
                                pv[dh][:],
                                id_sb[:],
                                acc[:, fin_ci, a:a + 512],
                                start=False, stop=True,
                                skip_group_check=True)

                def rs_dr(ci, pt, rs_last):
                    # row sums: one accumulation group spanning all windows
                    if 'rs' not in parts:
                        return
                    for gi, (jp2, role) in enumerate(
                            ((0, 0), (0, 1), (2, 0), (2, 1))):
                        nc.tensor.matmul(
                            rs_ps[:, ci:ci + 1],
                            pt[:, jp2:jp2 + 2, role, :],
                            ones_t[:, jp2:jp2 + 2, :],
                            start=(not rs_state['started']),
                            stop=(rs_last and gi == 3),
                            perf_mode=DR)
                        rs_state['started'] = True

                def pv_rs(W):
                    # windows >= 4 touch at most two l-chunks, so window
                    # pairs (4,5) and (6,7) accumulate directly in PSUM
                    ci0 = W // 2
                    pt = pt_tiles[W]
                    v_w = v_tiles[W]
                    paired = W >= 4
                    first = (not paired) or (W % 2 == 0)
                    last = (not paired) or (W % 2 == 1)
                    if 'pv' not in parts:
                        return
                    for ci in range(ci0, 4):
                        ptc = pt[:, :, :, ci * P:(ci + 1) * P]
                        rs_dr(ci, ptc, rs_last=(W == N_WIN - 1 and ci == 3))
                        if first:
                            pv = [pvps.tile([P, 512], f32,
                                            name=f"pv{W}_{ci}_{dh}", tag="pv")
                                  for dh in range(2)]
                            pv_pair[ci] = pv
                        else:
                            pv = pv_pair[ci]
                        fin = (last and W == 2 * ci + 1 and W != 0)
                        pv_dr(pv, ptc, v_w, first, last,
                              fin_ci=(ci if fin else None))
                        if not last:
                            continue
                        if W == 0:
                            for dh in range(2):
                                nc.vector.tensor_copy(
                                    acc[:, ci, dh * 512:dh * 512 + 512],
                                    pv[dh][:])
                        elif W == 2 * ci + 1 and ci == 3:
                            # last chunk: scale/store straight from PSUM in
                            # 256-col pieces; alternate DMA issue queues
                            nc.vector.reciprocal(recip_sb[:, 3:4],
                                                 rs_ps[:, 3:4])
                            o_t = outp.tile([P, D], bf16, name="o3", tag="o")
                            for i, a in enumerate(range(0, D, 512)):
                                nc.scalar.activation(
                                    o_t[:, a:a + 512],
                                    pv[a // 512][:],
                                    mybir.ActivationFunctionType.Copy,
                                    scale=recip_sb[:, 3:4])
                                eng = nc.sync if i % 2 == 0 else nc.scalar
                                eng.dma_start(
                                    bass.AP(outd, 3 * P * D + a,
                                            [[D, P], [1, 512]]),
                                    o_t[:, a:a + 512])
                        elif W == 2 * ci + 1:
                            finalize_half(ci, 1, pv[1][:])
                            finalize_half(ci, 0, pv[0][:])
                        else:
                            for dh in range(2):
                                nc.vector.tensor_add(
                                    acc[:, ci, dh * 512:dh * 512 + 512],
                                    acc[:, ci, dh * 512:dh * 512 + 512],
                                    pv[dh][:])

                # software pipeline: PE does scores(W+1) before pv(W)
                dma_win_kt(0)
                dma_win_v(0)
                dma_win_kt(1)
                dma_win_v(1)
                scores(0)
                for W in range(N_WIN):
                    if W + 2 < N_WIN:
                        dma_win_kt(W + 2)
                        dma_win_v(W + 2)
                    if W + 1 < N_WIN:
                        scores(W + 1)
                    pv_rs(W)

    nc.compile()
    return nc


def _host_inputs(x, W_query, W_key, W_value):
    f8 = ml_dtypes.float8_e4m3

    def split8(a):
        hi = a.astype(f8)
        lo = (a - hi.astype(np.float32)).astype(f8)
        return hi, lo

    def wprep(W):
        # [feat, dcol] * 32 -> [P, 8 ko, 2 role, D] e4m3 pairs
        hi, lo = split8(np.ascontiguousarray(W * 32.0, dtype=np.float32))
        out = np.empty((P, 8, 2, D), f8)
        out[:, :, 0, :] = hi.reshape(8, P, D).transpose(1, 0, 2)
        out[:, :, 1, :] = lo.reshape(8, P, D).transpose(1, 0, 2)
        return out

    def xprep(rows):
        xt = np.ascontiguousarray(x[rows].T, dtype=np.float32)  # [feat, 512]
        hi, lo = split8(xt)
        out = np.empty((P, 8, 2, L), f8)
        out[:, :, 0, :] = hi.reshape(8, P, L).transpose(1, 0, 2)
        out[:, :, 1, :] = lo.reshape(8, P, L).transpose(1, 0, 2)
        return out

    wq_n, wk_n, wv_n = wprep(W_query), wprep(W_key), wprep(W_value)

    in_maps = []
    for c in range(N_CORES):
        # mask[jp, lb]: within the 16-col band at l = 64W + 16js + lb, the
        # key j = 512W + 128js + jp is valid iff jp <= 8*lb + c  (same for
        # every window W and key block js)
        jp = np.arange(P)[:, None]
        lb = np.arange(16)[None, :]
        mask = np.where(jp <= 8 * lb + c, 0.0, BIG_NEG).astype(np.float32)
        in_maps.append({
            "wq": wq_n, "wk": wk_n, "wv": wv_n,
            "xkv": xprep(np.arange(L * c, L * (c + 1))),
            "xq": xprep(np.arange(L) * 8 + c),
            "mask": np.ascontiguousarray(mask),
            "ident": np.eye(P, dtype=np.float32),
        })
    return in_maps


def kernel(x, W_query, W_key, W_value):
    from concourse.bass_utils import run_bass_kernel_spmd

    x = np.asarray(x, dtype=np.float32)
    W_query = np.asarray(W_query, dtype=np.float32)
    W_key = np.asarray(W_key, dtype=np.float32)
    W_value = np.asarray(W_value, dtype=np.float32)

    if "nc" not in _CACHE:
        _CACHE["nc"] = _build()
    nc = _CACHE["nc"]

    in_maps = _host_inputs(x, W_query, W_key, W_value)
    res = run_bass_kernel_spmd(nc, in_maps, core_ids=list(range(N_CORES)))

    out = np.empty((S, D), dtype=np.float32)
    for c in range(N_CORES):
        # device result is 32x the true output (V was pre-scaled by 32)
        out[np.arange(L) * 8 + c] = \
            res.results[c]["out"].astype(np.float32) / 32.0
    return out
